# revision 1
# baseline (speedup 1.0000x reference)
"""KAN-SE (squeeze-excite with 2-layer KAN MLP) Trainium2 kernel.

Full-input contract: kernel(**inputs) takes the complete (32, 512, 64, 64)
batch plus KAN weights, shards the batch across 8 NeuronCores (4 samples
per core, data-parallel, weights replicated), and returns the full output.

Per-core device program (pure SPMD, no collectives):
  for each of 4 samples:
    - load the sample's (512, 4096) pixels as 4 tiles of (128, 4096) f32,
      keep them resident in SBUF
    - per-channel mean via free-dim reduce  -> s (512,)
    - 2-layer KAN on s (B-spline bases via Cox-de-Boor on VectorE,
      einsums as tiny PE matmuls accumulating in PSUM, SiLU/Sigmoid on
      ScalarE) -> per-channel gate (512,)
    - scale the resident tiles by the gate and store

x is read exactly once (SBUF-resident between mean and scale), so HBM
traffic is the 2x minimum: 8 MiB in + 8 MiB out per sample per core.
"""

import numpy as np

# ---- problem constants (hardcoded per contract; do not read spec/reference) ----
B, C, H, W = 32, 512, 64, 64
HIDDEN = 64            # max(16, 512 // 8)
KB = 8                 # GRID_SIZE + SPLINE_ORDER = 5 + 3
NCORES = 8
NS = B // NCORES       # samples per core = 4
NG = C // 128          # channel groups of 128 = 4
HWPIX = H * W          # 4096

# gtab column layout: [G0(12) | -g_i for k=1(10) | g_{i+2} k=1(10)
#                      | -g_i k=2(9) | g_{i+3} k=2(9) | -g_i k=3(8) | g_{i+4} k=3(8)]
_GT_OFF = {"G0": 0, 1: (12, 22), 2: (32, 41), 3: (50, 58)}
_GT_W = 66


def _grid_tables(grid_row: np.ndarray):
    """Build the (128, 66) constant table + per-level reciprocal immediates
    from one row of the (uniform) grid."""
    g = np.asarray(grid_row, np.float64)
    assert g.shape == (12,)
    h = g[1] - g[0]
    tab = np.zeros((_GT_W,), np.float64)
    tab[0:12] = g
    rs = {}
    for k in (1, 2, 3):
        w = 11 - k
        aoff, coff = _GT_OFF[k]
        tab[aoff:aoff + w] = -g[:w]          # -g_i,      i = 0..10-k
        tab[coff:coff + w] = g[k + 1:12]     # g_{i+k+1}, i = 0..10-k
        rs[k] = float(np.float32(1.0 / (k * h)))
    full = np.tile(tab.astype(np.float32)[None, :], (128, 1))
    return np.ascontiguousarray(full), rs


def _host_prep(inputs):
    """Rearrange weights into the SBUF layouts the device program uses."""
    f32 = np.float32
    base_w1 = np.asarray(inputs["base_w1"], f32)      # (64, 512)
    spline_w1 = np.asarray(inputs["spline_w1"], f32)  # (64, 512, 8)
    scaler1 = np.asarray(inputs["scaler1"], f32)      # (64, 512)
    base_w2 = np.asarray(inputs["base_w2"], f32)      # (512, 64)
    spline_w2 = np.asarray(inputs["spline_w2"], f32)  # (512, 64, 8)
    scaler2 = np.asarray(inputs["scaler2"], f32)      # (512, 64)

    # w1t[p, g*64+o] = base_w1[o, 128g+p]
    w1t = base_w1.reshape(HIDDEN, NG, 128).transpose(2, 1, 0).reshape(128, NG * HIDDEN)
    # sw1[p, (g*8+k)*64+o] = (spline_w1*scaler1)[o, 128g+p, k]
    sw1 = (spline_w1 * scaler1[:, :, None]).reshape(HIDDEN, NG, 128, KB)
    sw1 = sw1.transpose(2, 1, 3, 0).reshape(128, NG * KB * HIDDEN)
    # w2t[p, o] = base_w2[o, p]
    w2t = base_w2.T
    # sw2[p, k*512+o] = (spline_w2*scaler2)[o, p, k]
    sw2 = (spline_w2 * scaler2[:, :, None]).transpose(1, 2, 0).reshape(HIDDEN, KB * C)

    gt1, rs1 = _grid_tables(np.asarray(inputs["grid1"], f32)[0])
    gt2, rs2 = _grid_tables(np.asarray(inputs["grid2"], f32)[0])

    tensors = {
        "w1t": np.ascontiguousarray(w1t, f32),
        "sw1": np.ascontiguousarray(sw1, f32),
        "w2t": np.ascontiguousarray(w2t, f32),
        "sw2": np.ascontiguousarray(sw2, f32),
        "gt1": gt1,
        "gt2": gt2,
    }
    return tensors, rs1, rs2


def _emit_bsplines(nc, mybir, pool, gt_sb, x_ap, out_ap, p, rs):
    """Cubic B-spline bases of x (one value per partition) -> out_ap (p, 8).

    Cox-de-Boor on VectorE with per-basis-index grid constants from gt_sb
    and uniform-knot reciprocals rs (immediates).
    """
    f32 = mybir.dt.float32
    Alu = mybir.AluOpType
    ge = pool.tile([128, 12], f32, tag="ge", bufs=4)
    # ge[:, i] = (g_i <= x)
    nc.vector.tensor_scalar(
        out=ge[:p], in0=gt_sb[:p, 0:12], scalar1=x_ap, scalar2=None, op0=Alu.is_le
    )
    bprev = pool.tile([128, 11], f32, tag="b0", bufs=4)
    nc.vector.tensor_tensor(bprev[:p], ge[:p, 0:11], ge[:p, 1:12], Alu.subtract)
    for k in (1, 2, 3):
        w = 11 - k
        aoff, coff = _GT_OFF[k]
        a_t = pool.tile([128, 10], f32, tag="bsA", bufs=4)
        c_t = pool.tile([128, 10], f32, tag="bsC", bufs=4)
        # A = (x - g_i) / (k h);  C = (g_{i+k+1} - x) / (k h)
        nc.vector.tensor_scalar(
            out=a_t[:p, :w], in0=gt_sb[:p, aoff:aoff + w], scalar1=x_ap,
            scalar2=rs[k], op0=Alu.add, op1=Alu.mult,
        )
        nc.vector.tensor_scalar(
            out=c_t[:p, :w], in0=gt_sb[:p, coff:coff + w], scalar1=x_ap,
            scalar2=rs[k], op0=Alu.subtract, op1=Alu.mult,
        )
        if k < 3:
            bnext = pool.tile([128, 10], f32, tag="bn", bufs=4)
            outp = bnext[:p, :w]
        else:
            outp = out_ap
        nc.vector.tensor_tensor(c_t[:p, :w], c_t[:p, :w], bprev[:p, 1:w + 1], Alu.mult)
        nc.vector.tensor_tensor(outp, a_t[:p, :w], bprev[:p, 0:w], Alu.mult)
        nc.vector.tensor_tensor(outp, outp, c_t[:p, :w], Alu.add)
        if k < 3:
            bprev = bnext


def _build_nc(rs1, rs2):
    import concourse.bacc as bacc
    import concourse.bass as bass  # noqa: F401
    import concourse.mybir as mybir
    from concourse.tile import TileContext

    f32 = mybir.dt.float32
    Alu = mybir.AluOpType
    Act = mybir.ActivationFunctionType
    AX = mybir.AxisListType

    # Bacc (not plain Bass): its compile() runs move_matmul_waits_to_ldweights
    # + generate_event_semaphores, which split multi-waits down to the 1-wait-
    # per-instruction TRN2 ISA limit that walrus enforces.
    nc = bacc.Bacc("TRN2", target_bir_lowering=False)
    x_d = nc.declare_dram_parameter("x", [NS, C, H, W], f32, isOutput=False)
    w1t_d = nc.declare_dram_parameter("w1t", [128, NG * HIDDEN], f32, isOutput=False)
    sw1_d = nc.declare_dram_parameter("sw1", [128, NG * KB * HIDDEN], f32, isOutput=False)
    w2t_d = nc.declare_dram_parameter("w2t", [HIDDEN, C], f32, isOutput=False)
    sw2_d = nc.declare_dram_parameter("sw2", [HIDDEN, KB * C], f32, isOutput=False)
    gt1_d = nc.declare_dram_parameter("gt1", [128, _GT_W], f32, isOutput=False)
    gt2_d = nc.declare_dram_parameter("gt2", [128, _GT_W], f32, isOutput=False)
    y_d = nc.declare_dram_parameter("y", [NS, C, H, W], f32, isOutput=True)

    with TileContext(nc) as tc:
        with (
            tc.tile_pool(name="consts", bufs=1) as cpool,
            tc.tile_pool(name="xdata", bufs=2 * NG) as xpool,
            tc.tile_pool(name="small", bufs=3) as spool,
            tc.tile_pool(name="bspl", bufs=1) as bpool,
            tc.tile_pool(name="psum", bufs=2, space="PSUM") as ppool,
        ):
            w1t_sb = cpool.tile([128, NG * HIDDEN], f32)
            nc.sync.dma_start(w1t_sb[:], w1t_d[:, :])
            sw1_sb = cpool.tile([128, NG * KB * HIDDEN], f32)
            nc.sync.dma_start(sw1_sb[:], sw1_d[:, :])
            w2t_sb = cpool.tile([HIDDEN, C], f32)
            nc.sync.dma_start(w2t_sb[:], w2t_d[:, :])
            sw2_sb = cpool.tile([HIDDEN, KB * C], f32)
            nc.sync.dma_start(sw2_sb[:], sw2_d[:, :])
            gt1_sb = cpool.tile([128, _GT_W], f32)
            nc.sync.dma_start(gt1_sb[:], gt1_d[:, :])
            gt2_sb = cpool.tile([128, _GT_W], f32)
            nc.sync.dma_start(gt2_sb[:], gt2_d[:, :])

            # Pre-touch every const tile on VectorE: the DMA-completion wait
            # lands on these throwaway copies, so later DVE consumers (notably
            # TensorScalarPtr ops, whose ISA format has a single wait slot)
            # never need a DMA wait of their own.
            touch = cpool.tile([128, 8], f32)
            for i, ct in enumerate((w1t_sb, sw1_sb, gt1_sb, gt2_sb)):
                nc.vector.tensor_copy(touch[:, i:i + 1], ct[:, 0:1])
            for i, ct in enumerate((w2t_sb, sw2_sb)):
                nc.vector.tensor_copy(touch[:HIDDEN, 4 + i:5 + i], ct[:, 0:1])
            # Same for TensorE: the LDWEIGHTS sub-instruction also has a single
            # wait slot, so absorb each weight tile's DMA wait into a throwaway
            # 1-column matmul before the real accumulation chains.
            pt_ps = ppool.tile([1, 4], f32, tag="pt")
            for i, ct in enumerate((w1t_sb, sw1_sb)):
                nc.tensor.matmul(pt_ps[0:1, i:i + 1], ct[:, 0:1], ct[:, 0:1],
                                 start=True, stop=True)
            for i, ct in enumerate((w2t_sb, sw2_sb)):
                nc.tensor.matmul(pt_ps[0:1, 2 + i:3 + i], ct[:HIDDEN, 0:1],
                                 ct[:HIDDEN, 0:1], start=True, stop=True)

            for n in range(NS):
                # ---- load sample, per-channel sums ----
                sT = spool.tile([128, NG], f32, tag="sT")
                xts = []
                for g in range(NG):
                    xt = xpool.tile([128, HWPIX], f32, tag="xt")
                    src = x_d[n, 128 * g:128 * (g + 1)].rearrange("p h w -> p (h w)")
                    nc.sync.dma_start(xt[:], src)
                    nc.vector.reduce_sum(sT[:, g:g + 1], xt[:], axis=AX.X)
                    xts.append(xt)
                # raw sums -> means
                nc.vector.tensor_scalar(
                    out=sT[:], in0=sT[:], scalar1=1.0 / HWPIX, scalar2=None,
                    op0=Alu.mult,
                )

                # ---- KAN layer 1: s (512,) -> h1 (64,) ----
                silu1 = spool.tile([128, NG], f32, tag="silu1")
                nc.scalar.activation(silu1[:], sT[:], Act.Silu)
                bf = spool.tile([128, NG * KB], f32, tag="bf")
                for g in range(NG):
                    _emit_bsplines(
                        nc, mybir, bpool, gt1_sb, sT[:, g:g + 1],
                        bf[:, KB * g:KB * (g + 1)], 128, rs1,
                    )
                ps1 = ppool.tile([HIDDEN, 1], f32, tag="ps1")
                mms = []
                for g in range(NG):
                    mms.append((w1t_sb[:, HIDDEN * g:HIDDEN * (g + 1)], silu1[:, g:g + 1]))
                for g in range(NG):
                    for k in range(KB):
                        col = HIDDEN * (KB * g + k)
                        mms.append((sw1_sb[:, col:col + HIDDEN], bf[:, KB * g + k:KB * g + k + 1]))
                for i, (lhsT, rhs) in enumerate(mms):
                    nc.tensor.matmul(
                        ps1[:], lhsT, rhs, start=(i == 0), stop=(i == len(mms) - 1)
                    )

                # ---- inter-layer SiLU, KAN layer 2: t (64,) -> (512,) ----
                t1 = spool.tile([HIDDEN, 1], f32, tag="t1")
                nc.scalar.activation(t1[:], ps1[:], Act.Silu)
                silu2 = spool.tile([HIDDEN, 1], f32, tag="silu2")
                nc.scalar.activation(silu2[:], t1[:], Act.Silu)
                b2f = spool.tile([HIDDEN, KB], f32, tag="b2f")
                _emit_bsplines(nc, mybir, bpool, gt2_sb, t1[:, 0:1], b2f[:], HIDDEN, rs2)

                ps2 = ppool.tile([128, NG], f32, tag="ps2")
                for og in range(NG):
                    mms2 = [(w2t_sb[:, 128 * og:128 * (og + 1)], silu2[:, 0:1])]
                    for k in range(KB):
                        col = C * k + 128 * og
                        mms2.append((sw2_sb[:, col:col + 128], b2f[:, k:k + 1]))
                    for i, (lhsT, rhs) in enumerate(mms2):
                        nc.tensor.matmul(
                            ps2[:, og:og + 1], lhsT, rhs,
                            start=(i == 0), stop=(i == len(mms2) - 1),
                        )

                gate = spool.tile([128, NG], f32, tag="gate")
                nc.scalar.activation(gate[:], ps2[:], Act.Sigmoid)

                # ---- scale resident tiles by the gate, store ----
                for g in range(NG):
                    nc.vector.tensor_scalar(
                        out=xts[g][:], in0=xts[g][:], scalar1=gate[:, g:g + 1],
                        scalar2=None, op0=Alu.mult,
                    )
                    dst = y_d[n, 128 * g:128 * (g + 1)].rearrange("p h w -> p (h w)")
                    nc.sync.dma_start(dst, xts[g][:])
    nc.compile()
    return nc


def _run(inputs, trace=False):
    from concourse.bass_utils import run_bass_kernel_spmd

    x = np.ascontiguousarray(np.asarray(inputs["x"], np.float32))
    assert x.shape == (B, C, H, W), x.shape
    tensors, rs1, rs2 = _host_prep(inputs)
    nc = _build_nc(rs1, rs2)
    in_maps = []
    for c in range(NCORES):
        m = {"x": np.ascontiguousarray(x[NS * c:NS * (c + 1)])}
        m.update(tensors)
        in_maps.append(m)
    res = run_bass_kernel_spmd(
        nc, in_maps, core_ids=list(range(NCORES)), trace=trace
    )
    out = np.concatenate([res.results[c]["y"] for c in range(NCORES)], axis=0)
    return out, res


def kernel(**inputs) -> np.ndarray:
    return _run(inputs)[0]



# revision 2
# speedup vs baseline: 1.6461x; 1.6461x over previous
"""KAN-SE (squeeze-excite with 2-layer KAN MLP) Trainium2 kernel.

Full-input contract: kernel(**inputs) takes the complete (32, 512, 64, 64)
batch plus KAN weights, shards the batch across 8 NeuronCores (4 samples
per core, data-parallel, weights replicated), and returns the full output.

v2 (fp16): the rel-err gate is 2e-2 and the fp32 pipeline measured 4e-7,
so precision is traded for bandwidth/throughput:
  - x/y move over HBM as fp16 (host casts in/out): 2x less DMA traffic
    and 2x DVE throughput on the big reduce/scale passes.
  - KAN weights + features are fp16 on the PE (fp32 matmul on TRN2 is
    emitted as 2 HW matmuls; fp16 is native). Verified l2 rel err 3.9e-4.
  - samples are KAN-processed in pairs (rhs gets 2 columns) to halve the
    LDWEIGHTS count again.
  - b-splines are evaluated on RAW channel sums against a grid pre-scaled
    by H*W, so no serial sum->mean pass exists; the SiLU feature is
    sum * (1/HW) * sigmoid(sum/HW) in one DVE op.
  - ScalarE only ever runs Sigmoid + Copy activations; SiLU everywhere is
    x*sigmoid(x) with the multiply on DVE (avoids act-table reloads).
  - the per-channel gate multiply alternates VectorE / ScalarE per tile so
    neither engine bottlenecks the stream.

Per-core HBM traffic: 16 MiB in + 16 MiB out (fp16), read-once/write-once.
"""

import numpy as np

# ---- problem constants (hardcoded per contract; do not read spec/reference) ----
B, C, H, W = 32, 512, 64, 64
HIDDEN = 64            # max(16, 512 // 8)
KB = 8                 # GRID_SIZE + SPLINE_ORDER = 5 + 3
NCORES = 8
NS = B // NCORES       # samples per core = 4
SB = 2                 # samples per KAN batch (pair)
NBATCH = NS // SB      # 2 batches per core
NG = C // 128          # channel groups of 128 = 4
HWPIX = H * W          # 4096
NF = KB + 1            # features per channel: silu + 8 spline bases

# gtab column layout: [G0(12) | -g_i for k=1(10) | g_{i+2} k=1(10)
#                      | -g_i k=2(9) | g_{i+3} k=2(9) | -g_i k=3(8) | g_{i+4} k=3(8)]
_GT_OFF = {"G0": 0, 1: (12, 22), 2: (32, 41), 3: (50, 58)}
_GT_W = 66


def _grid_tables(grid_row: np.ndarray, xscale: float):
    """Build the (128, 66) constant table + per-level reciprocal immediates
    from one row of the (uniform) grid, for inputs pre-multiplied by
    1/xscale (i.e. the table is the grid scaled by xscale)."""
    g = np.asarray(grid_row, np.float64) * xscale
    assert g.shape == (12,)
    h = g[1] - g[0]
    tab = np.zeros((_GT_W,), np.float64)
    tab[0:12] = g
    rs = {}
    for k in (1, 2, 3):
        w = 11 - k
        aoff, coff = _GT_OFF[k]
        tab[aoff:aoff + w] = -g[:w]          # -g_i,      i = 0..10-k
        tab[coff:coff + w] = g[k + 1:12]     # g_{i+k+1}, i = 0..10-k
        rs[k] = float(np.float32(1.0 / (k * h)))
    full = np.tile(tab.astype(np.float32)[None, :], (128, 1))
    return np.ascontiguousarray(full), rs


def _host_prep(inputs):
    """Rearrange weights into the SBUF layouts the device program uses."""
    f32, f16 = np.float32, np.float16
    base_w1 = np.asarray(inputs["base_w1"], f32)      # (64, 512)
    spline_w1 = np.asarray(inputs["spline_w1"], f32)  # (64, 512, 8)
    scaler1 = np.asarray(inputs["scaler1"], f32)      # (64, 512)
    base_w2 = np.asarray(inputs["base_w2"], f32)      # (512, 64)
    spline_w2 = np.asarray(inputs["spline_w2"], f32)  # (512, 64, 8)
    scaler2 = np.asarray(inputs["scaler2"], f32)      # (512, 64)

    # w1t[p, g*64+o] = base_w1[o, 128g+p]
    w1t = base_w1.reshape(HIDDEN, NG, 128).transpose(2, 1, 0).reshape(128, NG * HIDDEN)
    # sw1[p, (g*8+k)*64+o] = (spline_w1*scaler1)[o, 128g+p, k]
    sw1 = (spline_w1 * scaler1[:, :, None]).reshape(HIDDEN, NG, 128, KB)
    sw1 = sw1.transpose(2, 1, 3, 0).reshape(128, NG * KB * HIDDEN)
    # w2t[p, o] = base_w2[o, p]
    w2t = base_w2.T
    # sw2[p, k*512+o] = (spline_w2*scaler2)[o, p, k]
    sw2 = (spline_w2 * scaler2[:, :, None]).transpose(1, 2, 0).reshape(HIDDEN, KB * C)

    # layer-1 b-splines run on raw per-channel SUMS (not means): grid x HWPIX
    gt1, rs1 = _grid_tables(np.asarray(inputs["grid1"], f32)[0], float(HWPIX))
    gt2, rs2 = _grid_tables(np.asarray(inputs["grid2"], f32)[0], 1.0)

    tensors = {
        "w1t": np.ascontiguousarray(w1t, f16),
        "sw1": np.ascontiguousarray(sw1, f16),
        "w2t": np.ascontiguousarray(w2t, f16),
        "sw2": np.ascontiguousarray(sw2, f16),
        "gt1": gt1,
        "gt2": gt2,
    }
    return tensors, rs1, rs2


def _emit_bsplines(nc, mybir, pool, gt_sb, x_ap, out_ap, p, rs):
    """Cubic B-spline bases of x (one value per partition) -> out_ap (p, 8).

    Cox-de-Boor on VectorE with per-basis-index grid constants from gt_sb
    and uniform-knot reciprocals rs (immediates). out_ap may be strided;
    it is written at the final level (fp16 out is fine).
    """
    f32 = mybir.dt.float32
    Alu = mybir.AluOpType
    ge = pool.tile([128, 12], f32, tag="ge", bufs=4)
    # ge[:, i] = (g_i <= x)
    nc.vector.tensor_scalar(
        out=ge[:p], in0=gt_sb[:p, 0:12], scalar1=x_ap, scalar2=None, op0=Alu.is_le
    )
    bprev = pool.tile([128, 11], f32, tag="b0", bufs=4)
    nc.vector.tensor_tensor(bprev[:p], ge[:p, 0:11], ge[:p, 1:12], Alu.subtract)
    for k in (1, 2, 3):
        w = 11 - k
        aoff, coff = _GT_OFF[k]
        a_t = pool.tile([128, 10], f32, tag="bsA", bufs=4)
        c_t = pool.tile([128, 10], f32, tag="bsC", bufs=4)
        # A = (x - g_i) / (k h);  C = (g_{i+k+1} - x) / (k h)
        nc.vector.tensor_scalar(
            out=a_t[:p, :w], in0=gt_sb[:p, aoff:aoff + w], scalar1=x_ap,
            scalar2=rs[k], op0=Alu.add, op1=Alu.mult,
        )
        nc.vector.tensor_scalar(
            out=c_t[:p, :w], in0=gt_sb[:p, coff:coff + w], scalar1=x_ap,
            scalar2=rs[k], op0=Alu.subtract, op1=Alu.mult,
        )
        if k < 3:
            bnext = pool.tile([128, 10], f32, tag="bn", bufs=4)
            outp = bnext[:p, :w]
        else:
            outp = out_ap
        nc.vector.tensor_tensor(c_t[:p, :w], c_t[:p, :w], bprev[:p, 1:w + 1], Alu.mult)
        nc.vector.tensor_tensor(outp, a_t[:p, :w], bprev[:p, 0:w], Alu.mult)
        nc.vector.tensor_tensor(outp, outp, c_t[:p, :w], Alu.add)
        if k < 3:
            bprev = bnext


def _build_nc(rs1, rs2):
    import concourse.bacc as bacc
    import concourse.bass as bass  # noqa: F401
    import concourse.mybir as mybir
    from concourse.tile import TileContext

    f32 = mybir.dt.float32
    f16 = mybir.dt.float16
    Alu = mybir.AluOpType
    Act = mybir.ActivationFunctionType
    AX = mybir.AxisListType

    # Bacc (not plain Bass): its compile() runs move_matmul_waits_to_ldweights
    # + generate_event_semaphores, which split multi-waits down to the 1-wait-
    # per-instruction TRN2 ISA limit that walrus enforces.
    nc = bacc.Bacc("TRN2", target_bir_lowering=False)
    x_d = nc.declare_dram_parameter("x", [NS, C, H, W], f16, isOutput=False)
    w1t_d = nc.declare_dram_parameter("w1t", [128, NG * HIDDEN], f16, isOutput=False)
    sw1_d = nc.declare_dram_parameter("sw1", [128, NG * KB * HIDDEN], f16, isOutput=False)
    w2t_d = nc.declare_dram_parameter("w2t", [HIDDEN, C], f16, isOutput=False)
    sw2_d = nc.declare_dram_parameter("sw2", [HIDDEN, KB * C], f16, isOutput=False)
    gt1_d = nc.declare_dram_parameter("gt1", [128, _GT_W], f32, isOutput=False)
    gt2_d = nc.declare_dram_parameter("gt2", [128, _GT_W], f32, isOutput=False)
    y_d = nc.declare_dram_parameter("y", [NS, C, H, W], f16, isOutput=True)

    with TileContext(nc) as tc:
        with (
            tc.tile_pool(name="consts", bufs=1) as cpool,
            tc.tile_pool(name="xdata", bufs=2 * SB * NG) as xpool,
            tc.tile_pool(name="small", bufs=2) as spool,
            tc.tile_pool(name="bspl", bufs=1) as bpool,
            tc.tile_pool(name="psum", bufs=2, space="PSUM") as ppool,
        ):
            w1t_sb = cpool.tile([128, NG * HIDDEN], f16)
            nc.sync.dma_start(w1t_sb[:], w1t_d[:, :])
            sw1_sb = cpool.tile([128, NG * KB * HIDDEN], f16)
            nc.sync.dma_start(sw1_sb[:], sw1_d[:, :])
            w2t_sb = cpool.tile([HIDDEN, C], f16)
            nc.sync.dma_start(w2t_sb[:], w2t_d[:, :])
            sw2_sb = cpool.tile([HIDDEN, KB * C], f16)
            nc.sync.dma_start(sw2_sb[:], sw2_d[:, :])
            gt1_sb = cpool.tile([128, _GT_W], f32)
            nc.sync.dma_start(gt1_sb[:], gt1_d[:, :])
            gt2_sb = cpool.tile([128, _GT_W], f32)
            nc.sync.dma_start(gt2_sb[:], gt2_d[:, :])

            # Pre-touch every const tile on VectorE: the DMA-completion wait
            # lands on these throwaway copies, so later DVE consumers (notably
            # TensorScalarPtr ops, whose ISA format has a single wait slot)
            # never need a DMA wait of their own.
            touch = cpool.tile([128, 8], f32)
            for i, ct in enumerate((w1t_sb, sw1_sb, gt1_sb, gt2_sb)):
                nc.vector.tensor_copy(touch[:, i:i + 1], ct[:, 0:1])
            for i, ct in enumerate((w2t_sb, sw2_sb)):
                nc.vector.tensor_copy(touch[:HIDDEN, 4 + i:5 + i], ct[:, 0:1])
            # Same for TensorE: the LDWEIGHTS sub-instruction also has a single
            # wait slot, so absorb each weight tile's DMA wait into a throwaway
            # 1-column matmul before the real accumulation chains.
            pt_ps = ppool.tile([1, 4], f32, tag="pt")
            for i, ct in enumerate((w1t_sb, sw1_sb)):
                nc.tensor.matmul(pt_ps[0:1, i:i + 1], ct[:, 0:1], ct[:, 0:1],
                                 start=True, stop=True)
            for i, ct in enumerate((w2t_sb, sw2_sb)):
                nc.tensor.matmul(pt_ps[0:1, 2 + i:3 + i], ct[:HIDDEN, 0:1],
                                 ct[:HIDDEN, 0:1], start=True, stop=True)
            # ScalarE pre-touch: absorb gt-table DMA waits + pre-load the
            # Sigmoid act table outside the steady-state chain.
            nc.scalar.activation(touch[:, 6:7], gt1_sb[:, 0:1], Act.Sigmoid)
            nc.scalar.activation(touch[:, 7:8], gt2_sb[:, 0:1], Act.Sigmoid)

            scale_tog = 0  # alternates the gate-multiply between DVE/ScalarE
            for b in range(NBATCH):
                # ---- load the 2 samples' tiles, raw per-channel sums ----
                sT = spool.tile([128, SB * NG], f32, tag="sT")  # col = s*NG+g
                xts = {}
                for s in range(SB):
                    n = SB * b + s
                    for g in range(NG):
                        xt = xpool.tile([128, HWPIX], f16, tag="xt")
                        src = x_d[n, 128 * g:128 * (g + 1)].rearrange("p h w -> p (h w)")
                        nc.sync.dma_start(xt[:], src)
                        nc.vector.reduce_sum(
                            sT[:, NG * s + g:NG * s + g + 1], xt[:], axis=AX.X)
                        xts[(s, g)] = xt

                # ---- KAN layer 1 features, per (group, sample) ----
                # bf[g]: [128, 2*NF] fp16, col 2j+s = feature j of sample s
                #   j=0: silu(mean) = sum*(1/HW)*sigmoid(sum/HW); j>=1: bases
                sig1 = spool.tile([128, SB * NG], f32, tag="sig1")
                bfs = []
                for g in range(NG):
                    bf = spool.tile([128, SB * NF], f16, tag=f"bf{g}")
                    bfj = bf.rearrange("p (j s) -> p j s", s=SB)
                    for s in range(SB):
                        scol = sT[:, NG * s + g:NG * s + g + 1]
                        sgcol = sig1[:, NG * s + g:NG * s + g + 1]
                        nc.scalar.activation(sgcol, scol, Act.Sigmoid,
                                             scale=1.0 / HWPIX)
                        nc.vector.tensor_scalar(
                            out=bfj[:, 0, s:s + 1], in0=sgcol, scalar1=scol,
                            scalar2=1.0 / HWPIX, op0=Alu.mult, op1=Alu.mult)
                        _emit_bsplines(
                            nc, mybir, bpool, gt1_sb, scol,
                            bfj[:, 1:NF, s], 128, rs1)
                    bfs.append(bf)

                # ---- layer-1 matmuls: 9 features x 4 groups, rhs [128, 2] ----
                ps1 = ppool.tile([HIDDEN, SB], f32, tag="ps1")
                nmm = NG * NF
                i = 0
                for g in range(NG):
                    bf = bfs[g]
                    for j in range(NF):
                        if j == 0:
                            lhsT = w1t_sb[:, HIDDEN * g:HIDDEN * (g + 1)]
                        else:
                            col = HIDDEN * (KB * g + (j - 1))
                            lhsT = sw1_sb[:, col:col + HIDDEN]
                        nc.tensor.matmul(
                            ps1[:], lhsT, bf[:, SB * j:SB * (j + 1)],
                            start=(i == 0), stop=(i == nmm - 1))
                        i += 1

                # ---- inter-layer: t1 = silu(ps1); layer-2 features ----
                sg = spool.tile([HIDDEN, 2 * SB], f32, tag="sg2")
                t1 = spool.tile([HIDDEN, SB], f32, tag="t1")
                nc.scalar.activation(sg[:, 0:SB], ps1[:], Act.Sigmoid)
                nc.vector.tensor_tensor(t1[:], sg[:, 0:SB], ps1[:], Alu.mult)
                bf2 = spool.tile([HIDDEN, SB * NF], f16, tag="bf2")
                bf2j = bf2.rearrange("p (j s) -> p j s", s=SB)
                nc.scalar.activation(sg[:, SB:2 * SB], t1[:], Act.Sigmoid)
                nc.vector.tensor_tensor(
                    bf2j[:, 0, :], sg[:, SB:2 * SB], t1[:], Alu.mult)
                for s in range(SB):
                    _emit_bsplines(nc, mybir, bpool, gt2_sb, t1[:, s:s + 1],
                                   bf2j[:, 1:NF, s], HIDDEN, rs2)

                # ---- layer-2 matmuls: 9 features x 4 out-groups, rhs [64, 2] ----
                ps2 = ppool.tile([128, SB * NG], f32, tag="ps2")  # col = og*SB+s
                for og in range(NG):
                    for j in range(NF):
                        if j == 0:
                            lhsT = w2t_sb[:, 128 * og:128 * (og + 1)]
                        else:
                            col = C * (j - 1) + 128 * og
                            lhsT = sw2_sb[:, col:col + 128]
                        nc.tensor.matmul(
                            ps2[:, SB * og:SB * (og + 1)], lhsT,
                            bf2[:, SB * j:SB * (j + 1)],
                            start=(j == 0), stop=(j == NF - 1))

                gate = spool.tile([128, SB * NG], f32, tag="gate")
                nc.scalar.activation(gate[:], ps2[:], Act.Sigmoid)

                # ---- scale resident tiles by the gate, store ----
                for g in range(NG):
                    for s in range(SB):
                        n = SB * b + s
                        xt = xts[(s, g)]
                        gcol = gate[:, SB * g + s:SB * g + s + 1]
                        if scale_tog % 3 != 2:  # 2/3 on DVE (2x fp16), 1/3 ScalarE
                            nc.vector.tensor_scalar(
                                out=xt[:], in0=xt[:], scalar1=gcol,
                                scalar2=None, op0=Alu.mult)
                        else:
                            nc.scalar.activation(xt[:], xt[:], Act.Copy,
                                                 scale=gcol)
                        scale_tog += 1
                        dst = y_d[n, 128 * g:128 * (g + 1)].rearrange("p h w -> p (h w)")
                        nc.sync.dma_start(dst, xt[:])
    nc.compile()
    return nc


def _run(inputs, trace=False):
    from concourse.bass_utils import run_bass_kernel_spmd

    x = np.asarray(inputs["x"])
    assert x.shape == (B, C, H, W), x.shape
    x16 = np.ascontiguousarray(x.astype(np.float16))
    tensors, rs1, rs2 = _host_prep(inputs)
    nc = _build_nc(rs1, rs2)
    in_maps = []
    for c in range(NCORES):
        m = {"x": np.ascontiguousarray(x16[NS * c:NS * (c + 1)])}
        m.update(tensors)
        in_maps.append(m)
    res = run_bass_kernel_spmd(
        nc, in_maps, core_ids=list(range(NCORES)), trace=trace
    )
    out = np.concatenate([res.results[c]["y"] for c in range(NCORES)], axis=0)
    return out.astype(np.float32), res


def kernel(**inputs) -> np.ndarray:
    return _run(inputs)[0]


# revision 5
# speedup vs baseline: 2.1128x; 1.2835x over previous
"""KAN-SE (squeeze-excite with 2-layer KAN MLP) Trainium2 kernel.

Full-input contract: kernel(**inputs) takes the complete (32, 512, 64, 64)
batch plus KAN weights, shards the batch across 8 NeuronCores (4 samples
per core, data-parallel, weights replicated), and returns the full output.

The rel-err gate is 2e-2 (fp32 pipeline measured 4e-7), so precision is
traded for bandwidth/throughput (verified 3.9e-4 l2 end to end):
  - x/y move over HBM as fp16 (host casts): 2x less DMA traffic, 2x DVE
    throughput on the big per-tile passes.
  - KAN weights + features are fp16 on the PE; samples are KAN-processed
    in pairs (rhs gets 2 columns) to amortize LDWEIGHTS.

v3 keeps every big-engine off the critical path (v2's DMA trace showed
60us of idle waiting on a serial VectorE chain):
  - per-channel sums alternate between DVE (tensor_scalar x1.0 with
    accum_out) and ScalarE (Copy with accum_out), both in-place, instead
    of 16x serial 4.4us TENSOR_REDUCEs (fp16 gets no 2x on reduce).
  - the whole Cox-de-Boor recurrence is batched over (sample, group) with
    stride-0 broadcast APs: ~18 DVE ops per layer per batch instead of
    ~17 per (g,s) pair.  1/(k h) is folded into the grid tables, 1/HW
    into the layer-1 base weights, so no serial scaling passes exist.
  - b-splines run on RAW channel sums against a grid pre-scaled by H*W.
  - ScalarE only runs Sigmoid/Copy; SiLU = x*sigmoid(x) with the mult on
    DVE (no act-table reloads in steady state).
  - the gate multiply is split 12 DVE / 4 ScalarE.

Per-core HBM traffic: 16 MiB in + 16 MiB out (fp16), read-once/write-once.
"""

import numpy as np

# ---- problem constants (hardcoded per contract; do not read spec/reference) ----
B, C, H, W = 32, 512, 64, 64
HIDDEN = 64            # max(16, 512 // 8)
KB = 8                 # GRID_SIZE + SPLINE_ORDER = 5 + 3
GRID_SIZE = 5
GRID_RANGE = (-6.0, 6.0)
NCORES = 8
NS = B // NCORES       # samples per core = 4
SB = 2                 # samples per KAN batch (pair)
NBATCH = NS // SB      # 2 batches per core
NG = C // 128          # channel groups of 128 = 4
HWPIX = H * W          # 4096
NF = KB + 1            # features per channel: silu + 8 spline bases


def _grid_cols(grid_row: np.ndarray, xscale: float, nsg: int):
    """Packed per-(s,g)-replicated grid constant columns for the batched
    Cox-de-Boor recurrence, evaluated on inputs x' = x * xscale.

    Returns (cols, offsets): cols is (ncol,) float32; offsets maps
      'ge'   -> start of g_i * xscale,        width nsg*12
      (k,'a')-> start of -g_i / (k h),        width nsg*(11-k)
      (k,'c')-> start of  g_{i+k+1} / (k h),  width nsg*(11-k)
      'rs'   -> start of 1/(k h xscale) k=1..3, width 3
    """
    g = np.asarray(grid_row, np.float64)
    assert g.shape == (12,)
    h = g[1] - g[0]
    segs, offsets = [], {}
    pos = 0

    def add(key, vals):
        nonlocal pos
        offsets[key] = pos
        segs.append(vals.astype(np.float32))
        pos += vals.size

    add('ge', np.tile(g * xscale, nsg))
    for k in (1, 2, 3):
        w = 11 - k
        add((k, 'a'), np.tile(-g[:w] / (k * h), nsg))
        add((k, 'c'), np.tile(g[k + 1:12] / (k * h), nsg))
    add('rs', np.array([1.0 / (k * h * xscale) for k in (1, 2, 3)]))
    return np.concatenate(segs), offsets


def _host_prep(inputs):
    """Rearrange weights into the SBUF layouts the device program uses."""
    f32, f16 = np.float32, np.float16
    base_w1 = np.asarray(inputs["base_w1"], f32)      # (64, 512)
    spline_w1 = np.asarray(inputs["spline_w1"], f32)  # (64, 512, 8)
    scaler1 = np.asarray(inputs["scaler1"], f32)      # (64, 512)
    base_w2 = np.asarray(inputs["base_w2"], f32)      # (512, 64)
    spline_w2 = np.asarray(inputs["spline_w2"], f32)  # (512, 64, 8)
    scaler2 = np.asarray(inputs["scaler2"], f32)      # (512, 64)

    # layer-1 silu feature arrives as sum*sigmoid(sum/HW) = HW*silu(mean),
    # so fold 1/HW into the base weights.
    # w1t[p, g*64+o] = base_w1[o, 128g+p] / HWPIX
    w1t = (base_w1 / HWPIX).reshape(HIDDEN, NG, 128)
    w1t = w1t.transpose(2, 1, 0).reshape(128, NG * HIDDEN)
    # sw1[p, (g*8+k)*64+o] = (spline_w1*scaler1)[o, 128g+p, k]
    sw1 = (spline_w1 * scaler1[:, :, None]).reshape(HIDDEN, NG, 128, KB)
    sw1 = sw1.transpose(2, 1, 3, 0).reshape(128, NG * KB * HIDDEN)
    # w2t[p, o] = base_w2[o, p]
    w2t = base_w2.T
    # sw2[p, k*512+o] = (spline_w2*scaler2)[o, p, k]
    sw2 = (spline_w2 * scaler2[:, :, None]).transpose(1, 2, 0).reshape(HIDDEN, KB * C)

    # packed grid-constant table: layer1 (on raw sums, xscale=HW, replicated
    # over sg=8) then layer2 (xscale=1, sg=2)
    c1, off1 = _grid_cols(np.asarray(inputs["grid1"], f32)[0], float(HWPIX), SB * NG)
    c2, off2 = _grid_cols(np.asarray(inputs["grid2"], f32)[0], 1.0, SB)
    off2 = {k: v + c1.size for k, v in off2.items()}
    gtab = np.concatenate([c1, c2])
    gtab_full = np.ascontiguousarray(np.tile(gtab[None, :], (128, 1)))

    tensors = {
        "w1t": np.ascontiguousarray(w1t, f16),
        "sw1": np.ascontiguousarray(sw1, f16),
        "w2t": np.ascontiguousarray(w2t, f16),
        "sw2": np.ascontiguousarray(sw2, f16),
        "gtab": gtab_full,
    }
    return tensors, off1, off2, gtab.size


def _emit_bsplines_batched(nc, mybir, pool, gtab_sb, off, sT2, out_j, P, S, G):
    """Cubic B-spline bases for all S*G per-partition scalars at once.

    sT2:   AP [P, S, G] of the (pre-scaled) inputs, col-packed s*G+g.
    out_j: AP [P, S, G, 8] (may be strided, fp16) for the final bases.
    off:   column offsets into gtab_sb for this layer's constant segs.
    Grid constants are replicated per (s,g) host-side; x is broadcast with
    stride-0 APs, so each Cox-de-Boor level is one DVE op over ~S*G*11 elems.
    """
    f32 = mybir.dt.float32
    Alu = mybir.AluOpType
    SG = S * G

    def rep(key, w):
        o = off[key]
        return gtab_sb[:P, o:o + SG * w].rearrange(
            "p (s g i) -> p s g i", s=S, g=G)

    ge = pool.tile([128, S, G, 12], f32, tag=f"ge{P}", bufs=2)
    xb = sT2.rearrange("p s g -> p s g ()")
    nc.vector.tensor_tensor(
        ge[:P], rep('ge', 12), xb.broadcast_to([P, S, G, 12]), Alu.is_le)
    bprev = pool.tile([128, S, G, 11], f32, tag=f"b0{P}", bufs=2)
    nc.vector.tensor_tensor(
        bprev[:P], ge[:P, :, :, 0:11], ge[:P, :, :, 1:12], Alu.subtract)
    # xr[p, k, s, g] = x * 1/(k h xscale)
    xr = pool.tile([128, 3, S, G], f32, tag=f"xr{P}", bufs=2)
    o = off['rs']
    rs_ap = gtab_sb[:P, o:o + 3].rearrange("p k -> p k () ()")
    nc.vector.tensor_tensor(
        xr[:P], rs_ap.broadcast_to([P, 3, S, G]),
        sT2.rearrange("p s g -> p () s g").broadcast_to([P, 3, S, G]), Alu.mult)
    for k in (1, 2, 3):
        w = 11 - k
        xk = xr[:P, k - 1].rearrange("p s g -> p s g ()").broadcast_to([P, S, G, w])
        a_t = pool.tile([128, S, G, 10], f32, tag=f"bsA{P}", bufs=2)
        c_t = pool.tile([128, S, G, 10], f32, tag=f"bsC{P}", bufs=2)
        # A = (x - g_i)/(k h) = xr + (-g_i/(k h));  C = g_{i+k+1}/(k h) - xr
        nc.vector.tensor_tensor(a_t[:P, :, :, :w], rep((k, 'a'), w), xk, Alu.add)
        nc.vector.tensor_tensor(c_t[:P, :, :, :w], rep((k, 'c'), w), xk,
                                Alu.subtract)
        if k < 3:
            bnext = pool.tile([128, S, G, 10], f32, tag=f"bn{P}", bufs=2)
            outp = bnext[:P, :, :, :w]
        else:
            outp = out_j
        nc.vector.tensor_tensor(
            c_t[:P, :, :, :w], c_t[:P, :, :, :w], bprev[:P, :, :, 1:w + 1], Alu.mult)
        nc.vector.tensor_tensor(outp, a_t[:P, :, :, :w], bprev[:P, :, :, 0:w], Alu.mult)
        nc.vector.tensor_tensor(outp, outp, c_t[:P, :, :, :w], Alu.add)
        if k < 3:
            bprev = bnext


def _build_nc(off1, off2, gtab_cols):
    import concourse.bacc as bacc
    import concourse.bass as bass  # noqa: F401
    import concourse.mybir as mybir
    from concourse.tile import TileContext

    f32 = mybir.dt.float32
    f16 = mybir.dt.float16
    Alu = mybir.AluOpType
    Act = mybir.ActivationFunctionType

    # Bacc (not plain Bass): its compile() runs move_matmul_waits_to_ldweights
    # + generate_event_semaphores, which split multi-waits down to the 1-wait-
    # per-instruction TRN2 ISA limit that walrus enforces.
    nc = bacc.Bacc("TRN2", target_bir_lowering=False)
    x_d = nc.declare_dram_parameter("x", [NS, C, H, W], f16, isOutput=False)
    w1t_d = nc.declare_dram_parameter("w1t", [128, NG * HIDDEN], f16, isOutput=False)
    sw1_d = nc.declare_dram_parameter("sw1", [128, NG * KB * HIDDEN], f16, isOutput=False)
    w2t_d = nc.declare_dram_parameter("w2t", [HIDDEN, C], f16, isOutput=False)
    sw2_d = nc.declare_dram_parameter("sw2", [HIDDEN, KB * C], f16, isOutput=False)
    gtab_d = nc.declare_dram_parameter("gtab", [128, gtab_cols], f32, isOutput=False)
    y_d = nc.declare_dram_parameter("y", [NS, C, H, W], f16, isOutput=True)

    with TileContext(nc) as tc:
        with (
            tc.tile_pool(name="consts", bufs=1) as cpool,
            tc.tile_pool(name="xdata", bufs=2 * SB * NG) as xpool,
            tc.tile_pool(name="small", bufs=2) as spool,
            tc.tile_pool(name="bspl", bufs=1) as bpool,
            tc.tile_pool(name="psum", bufs=2, space="PSUM") as ppool,
        ):
            w1t_sb = cpool.tile([128, NG * HIDDEN], f16)
            nc.sync.dma_start(w1t_sb[:], w1t_d[:, :])
            sw1_sb = cpool.tile([128, NG * KB * HIDDEN], f16)
            nc.sync.dma_start(sw1_sb[:], sw1_d[:, :])
            w2t_sb = cpool.tile([HIDDEN, C], f16)
            nc.sync.dma_start(w2t_sb[:], w2t_d[:, :])
            sw2_sb = cpool.tile([HIDDEN, KB * C], f16)
            nc.sync.dma_start(sw2_sb[:], sw2_d[:, :])
            gtab_sb = cpool.tile([128, gtab_cols], f32)
            nc.sync.dma_start(gtab_sb[:], gtab_d[:, :])

            # Pre-touch every const tile on VectorE: the DMA-completion wait
            # lands on these throwaway copies, so later DVE consumers (notably
            # TensorScalar ops, whose ISA format has a single wait slot)
            # never need a DMA wait of their own.
            touch = cpool.tile([128, 8], f32)
            for i, ct in enumerate((w1t_sb, sw1_sb, gtab_sb)):
                nc.vector.tensor_copy(touch[:, i:i + 1], ct[:, 0:1])
            for i, ct in enumerate((w2t_sb, sw2_sb)):
                nc.vector.tensor_copy(touch[:HIDDEN, 3 + i:4 + i], ct[:, 0:1])
            # Same for TensorE (LDWEIGHTS single wait slot).
            pt_ps = ppool.tile([1, 4], f32, tag="pt")
            for i, ct in enumerate((w1t_sb, sw1_sb)):
                nc.tensor.matmul(pt_ps[0:1, i:i + 1], ct[:, 0:1], ct[:, 0:1],
                                 start=True, stop=True)
            for i, ct in enumerate((w2t_sb, sw2_sb)):
                nc.tensor.matmul(pt_ps[0:1, 2 + i:3 + i], ct[:HIDDEN, 0:1],
                                 ct[:HIDDEN, 0:1], start=True, stop=True)
            # ScalarE: absorb the gtab DMA wait + pre-load the Sigmoid table.
            nc.scalar.activation(touch[:, 5:6], gtab_sb[:, 0:1], Act.Sigmoid)

            scale_tog = 0
            for b in range(NBATCH):
                # ---- load the 2 samples' tiles; raw per-channel sums ----
                # sums alternate DVE (tensor_scalar x1 + accum_out) and
                # ScalarE (Copy + accum_out), both in place: a plain DVE
                # reduce runs at the f32 rate and 16 of them serialize.
                sT = spool.tile([128, SB * NG], f32, tag="sT")  # col = s*NG+g
                xts = {}
                for s in range(SB):
                    n = SB * b + s
                    for g in range(NG):
                        xt = xpool.tile([128, HWPIX], f16, tag="xt")
                        src = x_d[n, 128 * g:128 * (g + 1)].rearrange("p h w -> p (h w)")
                        nc.sync.dma_start(xt[:], src)
                        scol = sT[:, NG * s + g:NG * s + g + 1]
                        if (NG * s + g) % 2 == 0:
                            nc.vector.tensor_scalar(
                                out=xt[:], in0=xt[:], scalar1=1.0, scalar2=None,
                                op0=Alu.mult, op1=Alu.add, accum_out=scol)
                        else:
                            nc.scalar.activation(xt[:], xt[:], Act.Copy,
                                                 accum_out=scol)
                        xts[(s, g)] = xt

                # ---- layer-1 features for all (s,g) at once ----
                # bft col = j*8 + s*4 + g, fp16; j=0 is the silu feature
                # sum*sigmoid(sum/HW) (the 1/HW lives in w1t).
                sig1 = spool.tile([128, SB * NG], f32, tag="sig1")
                nc.scalar.activation(sig1[:], sT[:], Act.Sigmoid, scale=1.0 / HWPIX)
                bft = spool.tile([128, NF * SB * NG], f16, tag="bft")
                bft4 = bft.rearrange("p (j s g) -> p j s g", s=SB, g=NG)
                sT2 = sT.rearrange("p (s g) -> p s g", g=NG)
                nc.vector.tensor_tensor(
                    bft4[:, 0], sig1.rearrange("p (s g) -> p s g", g=NG), sT2,
                    Alu.mult)
                _emit_bsplines_batched(
                    nc, mybir, bpool, gtab_sb, off1, sT2,
                    bft.rearrange("p (j s g) -> p s g j", s=SB, g=NG)[:, :, :, 1:NF],
                    128, SB, NG)

                # ---- layer-1 matmuls: 9 features x 4 groups, rhs [128, 2] ----
                ps1 = ppool.tile([HIDDEN, SB], f32, tag="ps1")
                nmm = NG * NF
                i = 0
                for g in range(NG):
                    for j in range(NF):
                        if j == 0:
                            lhsT = w1t_sb[:, HIDDEN * g:HIDDEN * (g + 1)]
                        else:
                            col = HIDDEN * (KB * g + (j - 1))
                            lhsT = sw1_sb[:, col:col + HIDDEN]
                        nc.tensor.matmul(
                            ps1[:], lhsT, bft4[:, j, :, g],
                            start=(i == 0), stop=(i == nmm - 1))
                        i += 1

                # ---- inter-layer: t1 = silu(ps1); layer-2 features ----
                sg = spool.tile([HIDDEN, 2 * SB], f32, tag="sg2")
                t1 = spool.tile([HIDDEN, SB], f32, tag="t1")
                nc.scalar.activation(sg[:, 0:SB], ps1[:], Act.Sigmoid)
                nc.vector.tensor_tensor(t1[:], sg[:, 0:SB], ps1[:], Alu.mult)
                bf2 = spool.tile([HIDDEN, NF * SB], f16, tag="bf2")
                bf24 = bf2.rearrange("p (j s g) -> p j s g", s=SB, g=1)
                nc.scalar.activation(sg[:, SB:2 * SB], t1[:], Act.Sigmoid)
                nc.vector.tensor_tensor(
                    bf24[:, 0, :, 0], sg[:, SB:2 * SB], t1[:], Alu.mult)
                _emit_bsplines_batched(
                    nc, mybir, bpool, gtab_sb, off2,
                    t1.rearrange("p (s g) -> p s g", g=1),
                    bf2.rearrange("p (j s g) -> p s g j", s=SB, g=1)[:, :, :, 1:NF],
                    HIDDEN, SB, 1)

                # ---- layer-2 matmuls: 9 features x 4 out-groups, rhs [64, 2] ----
                ps2 = ppool.tile([128, SB * NG], f32, tag="ps2")  # col = og*SB+s
                for og in range(NG):
                    for j in range(NF):
                        if j == 0:
                            lhsT = w2t_sb[:, 128 * og:128 * (og + 1)]
                        else:
                            col = C * (j - 1) + 128 * og
                            lhsT = sw2_sb[:, col:col + 128]
                        nc.tensor.matmul(
                            ps2[:, SB * og:SB * (og + 1)], lhsT, bf24[:, j, :, 0],
                            start=(j == 0), stop=(j == NF - 1))

                gate = spool.tile([128, SB * NG], f32, tag="gate")
                nc.scalar.activation(gate[:], ps2[:], Act.Sigmoid)

                # ---- scale resident tiles by the gate, store ----
                for g in range(NG):
                    for s in range(SB):
                        n = SB * b + s
                        xt = xts[(s, g)]
                        gcol = gate[:, SB * g + s:SB * g + s + 1]
                        if scale_tog % 4 != 3:  # 3/4 on DVE (2x fp16)
                            nc.vector.tensor_scalar(
                                out=xt[:], in0=xt[:], scalar1=gcol,
                                scalar2=None, op0=Alu.mult)
                        else:
                            nc.scalar.activation(xt[:], xt[:], Act.Copy,
                                                 scale=gcol)
                        scale_tog += 1
                        dst = y_d[n, 128 * g:128 * (g + 1)].rearrange("p h w -> p (h w)")
                        nc.sync.dma_start(dst, xt[:])
    nc.compile()
    return nc


def _run(inputs, trace=False):
    from concourse.bass_utils import run_bass_kernel_spmd

    x = np.asarray(inputs["x"])
    assert x.shape == (B, C, H, W), x.shape
    x16 = np.ascontiguousarray(x.astype(np.float16))
    tensors, off1, off2, gtab_cols = _host_prep(inputs)
    nc = _build_nc(off1, off2, gtab_cols)
    in_maps = []
    for c in range(NCORES):
        m = {"x": np.ascontiguousarray(x16[NS * c:NS * (c + 1)])}
        m.update(tensors)
        in_maps.append(m)
    res = run_bass_kernel_spmd(
        nc, in_maps, core_ids=list(range(NCORES)), trace=trace
    )
    out = np.concatenate([res.results[c]["y"] for c in range(NCORES)], axis=0)
    return out.astype(np.float32), res


def kernel(**inputs) -> np.ndarray:
    return _run(inputs)[0]


# revision 8
# speedup vs baseline: 2.3861x; 1.1293x over previous
"""KAN-SE (squeeze-excite with 2-layer KAN MLP) Trainium2 kernel.

Full-input contract: kernel(**inputs) takes the complete (32, 512, 64, 64)
batch plus KAN weights, shards the batch across 8 NeuronCores (4 samples
per core, data-parallel, weights replicated), and returns the full output.

The rel-err gate is 2e-2 (fp32 pipeline measured 4e-7), so precision is
traded for bandwidth/throughput (verified ~4e-4 l2 end to end): x/y move
over HBM as fp16 (host casts both ways), KAN weights/features are fp16 on
the PE, intermediate sums/activations stay f32.

v4 structure (from v3's trace: stores only overlap loads when gates come
early, and per-channel sums must not serialize on one engine):
  - all 16 tile loads are emitted first (doorbells ahead of everything),
    each followed by its row-sum on a rotating engine:
      DVE:    tensor_tensor_reduce(half0 + half1, accum add) - one op
      ScalarE: Copy activation with accum_out, in place
      GpSimd:  tensor_scalar x1.0 with accum_out, in place
    so sums keep pace with the ~3.3us/tile load rate.
  - the KAN runs per sample (4 small batches): the first gate exists
    ~25us in, so stores stream concurrently with the remaining loads.
  - b-splines on RAW sums vs a grid pre-scaled by H*W (no mean pass);
    whole Cox-de-Boor level batched over groups via stride-0 APs;
    1/(k h) folded into grid tables, 1/HW folded into layer-1 base w.
  - ScalarE only runs Sigmoid/Copy (SiLU = x*sigmoid(x), mult on DVE).
  - gate multiplies mostly on DVE (fp16 hits the 4x DVE mode, ~1.3us).

Per-core HBM traffic: 16 MiB in + 16 MiB out (fp16), read-once/write-once.
"""

import numpy as np

# ---- problem constants (hardcoded per contract; do not read spec/reference) ----
B, C, H, W = 32, 512, 64, 64
HIDDEN = 64            # max(16, 512 // 8)
KB = 8                 # GRID_SIZE + SPLINE_ORDER = 5 + 3
NCORES = 8
NS = B // NCORES       # samples per core = 4
NG = C // 128          # channel groups of 128 = 4
HWPIX = H * W          # 4096
NF = KB + 1            # features per channel: silu + 8 spline bases

# row-sum engine per tile index t = n*4+g (V=DVE ttr, S=ScalarE);
# GpSimd can't run tensor ops (ISA check rejects them on Pool)
SUM_ENG = "VSVS" "VSVS" "VSVS" "VSVS"
# gate-multiply engine per tile index (DVE 4x fp16 is ~3x faster than ScalarE)
SCALE_ENG = "VVSV" "VSVV" "VVVV" "VVVV"


def _grid_cols(grid_row: np.ndarray, xscale: float, nsg: int):
    """Packed per-group-replicated grid constant columns for the batched
    Cox-de-Boor recurrence, evaluated on inputs x' = x * xscale.

    offsets maps:
      'ge'   -> start of g_i * xscale,        width nsg*12
      (k,'a')-> start of -g_i / (k h),        width nsg*(11-k)
      (k,'c')-> start of  g_{i+k+1} / (k h),  width nsg*(11-k)
      'rs'   -> start of 1/(k h xscale), k=1..3
    """
    g = np.asarray(grid_row, np.float64)
    assert g.shape == (12,)
    h = g[1] - g[0]
    segs, offsets = [], {}
    pos = 0

    def add(key, vals):
        nonlocal pos
        offsets[key] = pos
        segs.append(vals.astype(np.float32))
        pos += vals.size

    add('ge', np.tile(g * xscale, nsg))
    for k in (1, 2, 3):
        w = 11 - k
        add((k, 'a'), np.tile(-g[:w] / (k * h), nsg))
        add((k, 'c'), np.tile(g[k + 1:12] / (k * h), nsg))
    add('rs', np.array([1.0 / (k * h * xscale) for k in (1, 2, 3)]))
    return np.concatenate(segs), offsets


def _host_prep(inputs):
    """Rearrange weights into the SBUF layouts the device program uses."""
    f32, f16 = np.float32, np.float16
    base_w1 = np.asarray(inputs["base_w1"], f32)      # (64, 512)
    spline_w1 = np.asarray(inputs["spline_w1"], f32)  # (64, 512, 8)
    scaler1 = np.asarray(inputs["scaler1"], f32)      # (64, 512)
    base_w2 = np.asarray(inputs["base_w2"], f32)      # (512, 64)
    spline_w2 = np.asarray(inputs["spline_w2"], f32)  # (512, 64, 8)
    scaler2 = np.asarray(inputs["scaler2"], f32)      # (512, 64)

    # layer-1 silu feature arrives as sum*sigmoid(sum/HW) = HW*silu(mean),
    # so fold 1/HW into the base weights.
    # w1t[p, g*64+o] = base_w1[o, 128g+p] / HWPIX
    w1t = (base_w1 / HWPIX).reshape(HIDDEN, NG, 128)
    w1t = w1t.transpose(2, 1, 0).reshape(128, NG * HIDDEN)
    # sw1[p, (g*8+k)*64+o] = (spline_w1*scaler1)[o, 128g+p, k]
    sw1 = (spline_w1 * scaler1[:, :, None]).reshape(HIDDEN, NG, 128, KB)
    sw1 = sw1.transpose(2, 1, 3, 0).reshape(128, NG * KB * HIDDEN)
    # w2t[p, o] = base_w2[o, p]
    w2t = base_w2.T
    # sw2[p, k*512+o] = (spline_w2*scaler2)[o, p, k]
    sw2 = (spline_w2 * scaler2[:, :, None]).transpose(1, 2, 0).reshape(HIDDEN, KB * C)

    # packed grid-constant table: layer1 (on raw sums, xscale=HW, replicated
    # over the 4 groups) then layer2 (xscale=1, single copy)
    c1, off1 = _grid_cols(np.asarray(inputs["grid1"], f32)[0], float(HWPIX), NG)
    c2, off2 = _grid_cols(np.asarray(inputs["grid2"], f32)[0], 1.0, 1)
    off2 = {k: v + c1.size for k, v in off2.items()}
    gtab = np.concatenate([c1, c2])
    gtab_full = np.ascontiguousarray(np.tile(gtab[None, :], (128, 1)))

    tensors = {
        "w1t": np.ascontiguousarray(w1t, f16),
        "sw1": np.ascontiguousarray(sw1, f16),
        "w2t": np.ascontiguousarray(w2t, f16),
        "sw2": np.ascontiguousarray(sw2, f16),
        "gtab": gtab_full,
    }
    return tensors, off1, off2, gtab.size


def _emit_bsplines_batched(nc, mybir, pool, gtab_sb, off, sT2, out_j, P, G):
    """Cubic B-spline bases for all G per-partition scalars at once.

    sT2:   AP [P, G] of the (pre-scaled) inputs.
    out_j: AP [P, G, 8] (may be strided, fp16) for the final bases.
    Grid constants are replicated per group host-side; x is broadcast with
    stride-0 APs, so each Cox-de-Boor level is one DVE op over ~G*11 elems.
    """
    f32 = mybir.dt.float32
    Alu = mybir.AluOpType

    def rep(key, w):
        o = off[key]
        return gtab_sb[:P, o:o + G * w].rearrange("p (g i) -> p g i", g=G)

    ge = pool.tile([128, G, 12], f32, tag=f"ge{P}", bufs=2)
    xb = sT2.rearrange("p g -> p g ()")
    nc.vector.tensor_tensor(
        ge[:P], rep('ge', 12), xb.broadcast_to([P, G, 12]), Alu.is_le)
    bprev = pool.tile([128, G, 11], f32, tag=f"b0{P}", bufs=2)
    nc.vector.tensor_tensor(
        bprev[:P], ge[:P, :, 0:11], ge[:P, :, 1:12], Alu.subtract)
    # xr[p, k, g] = x * 1/(k h xscale)
    xr = pool.tile([128, 3, G], f32, tag=f"xr{P}", bufs=2)
    o = off['rs']
    rs_ap = gtab_sb[:P, o:o + 3].rearrange("p k -> p k ()")
    nc.vector.tensor_tensor(
        xr[:P], rs_ap.broadcast_to([P, 3, G]),
        sT2.rearrange("p g -> p () g").broadcast_to([P, 3, G]), Alu.mult)
    for k in (1, 2, 3):
        w = 11 - k
        xk = xr[:P, k - 1].rearrange("p g -> p g ()").broadcast_to([P, G, w])
        a_t = pool.tile([128, G, 10], f32, tag=f"bsA{P}", bufs=2)
        c_t = pool.tile([128, G, 10], f32, tag=f"bsC{P}", bufs=2)
        # A = (x - g_i)/(k h) = xr + (-g_i/(k h));  C = g_{i+k+1}/(k h) - xr
        nc.vector.tensor_tensor(a_t[:P, :, :w], rep((k, 'a'), w), xk, Alu.add)
        nc.vector.tensor_tensor(c_t[:P, :, :w], rep((k, 'c'), w), xk, Alu.subtract)
        if k < 3:
            bnext = pool.tile([128, G, 10], f32, tag=f"bn{P}", bufs=2)
            outp = bnext[:P, :, :w]
        else:
            outp = out_j
        nc.vector.tensor_tensor(
            c_t[:P, :, :w], c_t[:P, :, :w], bprev[:P, :, 1:w + 1], Alu.mult)
        nc.vector.tensor_tensor(outp, a_t[:P, :, :w], bprev[:P, :, 0:w], Alu.mult)
        nc.vector.tensor_tensor(outp, outp, c_t[:P, :, :w], Alu.add)
        if k < 3:
            bprev = bnext


def _build_nc(off1, off2, gtab_cols):
    import concourse.bacc as bacc
    import concourse.bass as bass  # noqa: F401
    import concourse.mybir as mybir
    from concourse.tile import TileContext

    f32 = mybir.dt.float32
    f16 = mybir.dt.float16
    Alu = mybir.AluOpType
    Act = mybir.ActivationFunctionType

    # Bacc (not plain Bass): its compile() runs move_matmul_waits_to_ldweights
    # + generate_event_semaphores, which split multi-waits down to the 1-wait-
    # per-instruction TRN2 ISA limit that walrus enforces.
    nc = bacc.Bacc("TRN2", target_bir_lowering=False)
    x_d = nc.declare_dram_parameter("x", [NS, C, H, W], f16, isOutput=False)
    w1t_d = nc.declare_dram_parameter("w1t", [128, NG * HIDDEN], f16, isOutput=False)
    sw1_d = nc.declare_dram_parameter("sw1", [128, NG * KB * HIDDEN], f16, isOutput=False)
    w2t_d = nc.declare_dram_parameter("w2t", [HIDDEN, C], f16, isOutput=False)
    sw2_d = nc.declare_dram_parameter("sw2", [HIDDEN, KB * C], f16, isOutput=False)
    gtab_d = nc.declare_dram_parameter("gtab", [128, gtab_cols], f32, isOutput=False)
    y_d = nc.declare_dram_parameter("y", [NS, C, H, W], f16, isOutput=True)

    with TileContext(nc) as tc:
        with (
            tc.tile_pool(name="consts", bufs=1) as cpool,
            tc.tile_pool(name="xdata", bufs=NS * NG) as xpool,
            tc.tile_pool(name="small", bufs=NS) as spool,
            tc.tile_pool(name="bspl", bufs=1) as bpool,
            tc.tile_pool(name="psum", bufs=2, space="PSUM") as ppool,
        ):
            # ---- all 16 tile loads first; row sums on a rotating engine ----
            sTs = []
            xts = {}
            consts_emitted = False

            def emit_consts():
                w1t_sb = cpool.tile([128, NG * HIDDEN], f16)
                nc.sync.dma_start(w1t_sb[:], w1t_d[:, :])
                sw1_sb = cpool.tile([128, NG * KB * HIDDEN], f16)
                nc.sync.dma_start(sw1_sb[:], sw1_d[:, :])
                w2t_sb = cpool.tile([HIDDEN, C], f16)
                nc.sync.dma_start(w2t_sb[:], w2t_d[:, :])
                sw2_sb = cpool.tile([HIDDEN, KB * C], f16)
                nc.sync.dma_start(sw2_sb[:], sw2_d[:, :])
                gtab_sb = cpool.tile([128, gtab_cols], f32)
                nc.sync.dma_start(gtab_sb[:], gtab_d[:, :])
                # Pre-touch every const tile on VectorE: the DMA-completion
                # wait lands on these throwaway copies, so later DVE consumers
                # (notably TensorScalar ops, whose ISA format has a single
                # wait slot) never need a DMA wait of their own.
                touch = cpool.tile([128, 8], f32)
                for i, ct in enumerate((w1t_sb, sw1_sb, gtab_sb)):
                    nc.vector.tensor_copy(touch[:, i:i + 1], ct[:, 0:1])
                for i, ct in enumerate((w2t_sb, sw2_sb)):
                    nc.vector.tensor_copy(touch[:HIDDEN, 3 + i:4 + i], ct[:, 0:1])
                # Same for TensorE (LDWEIGHTS single wait slot).
                pt_ps = ppool.tile([1, 4], f32, tag="pt")
                for i, ct in enumerate((w1t_sb, sw1_sb)):
                    nc.tensor.matmul(pt_ps[0:1, i:i + 1], ct[:, 0:1], ct[:, 0:1],
                                     start=True, stop=True)
                for i, ct in enumerate((w2t_sb, sw2_sb)):
                    nc.tensor.matmul(pt_ps[0:1, 2 + i:3 + i], ct[:HIDDEN, 0:1],
                                     ct[:HIDDEN, 0:1], start=True, stop=True)
                # ScalarE: absorb the gtab DMA wait + pre-load Sigmoid table.
                nc.scalar.activation(touch[:, 5:6], gtab_sb[:, 0:1], Act.Sigmoid)
                return w1t_sb, sw1_sb, w2t_sb, sw2_sb, gtab_sb

            for n in range(NS):
                sT = spool.tile([128, NG], f32, tag="sT")
                sTs.append(sT)
                for g in range(NG):
                    t = NG * n + g
                    xt = xpool.tile([128, HWPIX], f16, tag="xt")
                    src = x_d[n, 128 * g:128 * (g + 1)].rearrange("p h w -> p (h w)")
                    nc.sync.dma_start(xt[:], src)
                    scol = sT[:, g:g + 1]
                    eng = SUM_ENG[t]
                    if eng == "V":
                        nc.vector.tensor_scalar(
                            out=xt[:], in0=xt[:], scalar1=1.0, scalar2=None,
                            op0=Alu.mult, op1=Alu.add, accum_out=scol)
                    elif eng == "S":
                        nc.scalar.activation(xt[:], xt[:], Act.Copy,
                                             accum_out=scol)
                    else:
                        nc.gpsimd.tensor_scalar(
                            out=xt[:], in0=xt[:], scalar1=1.0, scalar2=None,
                            op0=Alu.mult, op1=Alu.add, accum_out=scol)
                    xts[(n, g)] = xt
                if n == 1 and not consts_emitted:
                    consts = emit_consts()
                    consts_emitted = True
            w1t_sb, sw1_sb, w2t_sb, sw2_sb, gtab_sb = consts

            # ---- per-sample KAN + gate + scale + store ----
            for n in range(NS):
                sT = sTs[n]
                # layer-1 features; bft col = j*4 + g, fp16; j=0 is the silu
                # feature sum*sigmoid(sum/HW) (the 1/HW lives in w1t).
                sig1 = spool.tile([128, NG], f32, tag="sig1")
                nc.scalar.activation(sig1[:], sT[:], Act.Sigmoid, scale=1.0 / HWPIX)
                bft = spool.tile([128, NF * NG], f16, tag="bft")
                bft3 = bft.rearrange("p (j g) -> p j g", g=NG)
                nc.vector.tensor_tensor(bft3[:, 0], sig1[:], sT[:], Alu.mult)
                _emit_bsplines_batched(
                    nc, mybir, bpool, gtab_sb, off1, sT[:],
                    bft.rearrange("p (j g) -> p g j", g=NG)[:, :, 1:NF],
                    128, NG)

                # layer-1 matmuls: 9 features x 4 groups, rhs [128, 1]
                ps1 = ppool.tile([HIDDEN, 1], f32, tag="ps1")
                nmm = NG * NF
                i = 0
                for g in range(NG):
                    for j in range(NF):
                        if j == 0:
                            lhsT = w1t_sb[:, HIDDEN * g:HIDDEN * (g + 1)]
                        else:
                            col = HIDDEN * (KB * g + (j - 1))
                            lhsT = sw1_sb[:, col:col + HIDDEN]
                        nc.tensor.matmul(
                            ps1[:], lhsT, bft3[:, j, g:g + 1],
                            start=(i == 0), stop=(i == nmm - 1))
                        i += 1

                # inter-layer: t1 = silu(ps1); layer-2 features
                sg = spool.tile([HIDDEN, 2], f32, tag="sg2")
                t1 = spool.tile([HIDDEN, 1], f32, tag="t1")
                nc.scalar.activation(sg[:, 0:1], ps1[:], Act.Sigmoid)
                nc.vector.tensor_tensor(t1[:], sg[:, 0:1], ps1[:], Alu.mult)
                bf2 = spool.tile([HIDDEN, NF], f16, tag="bf2")
                nc.scalar.activation(sg[:, 1:2], t1[:], Act.Sigmoid)
                nc.vector.tensor_tensor(bf2[:, 0:1], sg[:, 1:2], t1[:], Alu.mult)
                _emit_bsplines_batched(
                    nc, mybir, bpool, gtab_sb, off2,
                    t1.rearrange("p one -> p one", one=1),
                    bf2.rearrange("p (j g) -> p g j", g=1)[:, :, 1:NF],
                    HIDDEN, 1)

                # layer-2 matmuls: 9 features x 4 out-groups, rhs [64, 1]
                ps2 = ppool.tile([128, NG], f32, tag="ps2")
                for og in range(NG):
                    for j in range(NF):
                        if j == 0:
                            lhsT = w2t_sb[:, 128 * og:128 * (og + 1)]
                        else:
                            col = C * (j - 1) + 128 * og
                            lhsT = sw2_sb[:, col:col + 128]
                        nc.tensor.matmul(
                            ps2[:, og:og + 1], lhsT, bf2[:, j:j + 1],
                            start=(j == 0), stop=(j == NF - 1))

                gate = spool.tile([128, NG], f32, tag="gate")
                nc.scalar.activation(gate[:], ps2[:], Act.Sigmoid)

                # scale resident tiles by the gate, store
                for g in range(NG):
                    xt = xts[(n, g)]
                    gcol = gate[:, g:g + 1]
                    if SCALE_ENG[NG * n + g] == "V":
                        nc.vector.tensor_scalar(
                            out=xt[:], in0=xt[:], scalar1=gcol,
                            scalar2=None, op0=Alu.mult)
                    else:
                        nc.scalar.activation(xt[:], xt[:], Act.Copy, scale=gcol)
                    dst = y_d[n, 128 * g:128 * (g + 1)].rearrange("p h w -> p (h w)")
                    nc.sync.dma_start(dst, xt[:])
    nc.compile()
    return nc


def _run(inputs, trace=False):
    from concourse.bass_utils import run_bass_kernel_spmd

    x = np.asarray(inputs["x"])
    assert x.shape == (B, C, H, W), x.shape
    x16 = np.ascontiguousarray(x.astype(np.float16))
    tensors, off1, off2, gtab_cols = _host_prep(inputs)
    nc = _build_nc(off1, off2, gtab_cols)
    in_maps = []
    for c in range(NCORES):
        m = {"x": np.ascontiguousarray(x16[NS * c:NS * (c + 1)])}
        m.update(tensors)
        in_maps.append(m)
    res = run_bass_kernel_spmd(
        nc, in_maps, core_ids=list(range(NCORES)), trace=trace
    )
    out = np.concatenate([res.results[c]["y"] for c in range(NCORES)], axis=0)
    return out.astype(np.float32), res


def kernel(**inputs) -> np.ndarray:
    return _run(inputs)[0]
